# revision 26
# baseline (speedup 1.0000x reference)
"""Trainium2 Bass kernel for nn_ExpandEvecs.

Computes, for evecs [B=4, C=1, N=1024, K=16]:
    outers[b,k,i,j] = evecs[b,0,i,k] * evecs[b,0,j,k]
    cube = cumsum(outers, axis=k)  ->  [B, K, N, N]
i.e. cube[b,l] = V[:, :l+1] @ V[:, :l+1]^T  (Gram expansion per level).

Every level is SYMMETRIC, so the device only computes the upper
block-triangle (56% of the elements; diagonal 128-blocks in full) and
the host mirrors the strictly-lower blocks during unsharding. Output
is stored bf16 and upcast on the host (2.4e-3 max rel err vs the 2e-2
gate). Per-core HBM stores drop to 9 MiB (~26 us at the ~358 GB/s
HBM-per-core limit), PE columns and PSUM->SBUF copy work drop by the
same 2.3x vs the full-matrix version.

Sharding: 8 cores = 4 batches x 2 triangle-halves. The upper
block-triangle of each [1024,1024] level splits into six 128-row
pieces per core with IDENTICAL shapes on both cores (SPMD-safe):
sizes (512,512,512,384,256,128) columns. Piece p of a core is
(block-row i_p, cols c0_p:c1_p); the host knows the same table.

Per core, per level: 6 bf16 matmuls (one per piece, 2304 PE columns
total) using the A/B split trick (V = A + B with A = bf16(V),
B = bf16(V-A); lhsT/rhs partition-interleaved so AA^T+AB^T+BA^T comes
out of one matmul with contraction 3*(l+1); the dropped BB^T term is
~2^-18 relative). Pieces pack pairwise into three PSUM tiles
([128,1024], [128,896], [128,384]) so each level needs only three
PSUM->SBUF bf16 cast copies, alternating Vector/Scalar by level
parity. One contiguous 576 KiB store per level ([128, 2304] tile,
4.5 KiB runs per partition).
"""

import numpy as np
import ml_dtypes

import concourse.mybir as mybir
from concourse import bacc, bass
from concourse.tile import TileContext
from concourse.bass_utils import run_bass_kernel_spmd

B, C, N, K = 4, 1, 1024, 16
NCORES = 8
K3 = 3 * K             # stacked contraction partitions
PACK = 2304            # packed free dim per level (1024+896+384)

F32 = mybir.dt.float32
BF16 = mybir.dt.bfloat16
BF16_NP = ml_dtypes.bfloat16

# pieces per core-half: (block_row, col0, col1); identical shape lists
# (512,512,512,384,256,128) on both halves (SPMD-safe).
PIECES = [
    [(0, 0, 512), (0, 512, 1024), (4, 512, 1024),
     (1, 640, 1024), (2, 768, 1024), (3, 896, 1024)],
    [(1, 128, 640), (2, 256, 768), (3, 384, 896),
     (5, 640, 1024), (6, 768, 1024), (7, 896, 1024)],
]
# PSUM banks are 512 f32; tiles round up to whole banks and only 8
# exist. Five one-bank segments let the three 512-piece tags run with
# bufs=2 (6 banks) so no serial level-chain forms through them; the two
# small segments (384+128 packed, and 256) tolerate bufs=1 chains.
# SEG: (pieces, packed offset, width, bufs), in issue order (chained
# small segments first so their copies start earliest each level).
SEG = [
    ((3, 5), 1536, 512, 1),
    ((4,), 2048, 256, 1),
    ((0,), 0, 512, 2),
    ((1,), 512, 512, 2),
    ((2,), 1024, 512, 2),
]
# packed offset of each piece (piece index -> offset)
POFFS = [0, 512, 1024, 1536, 2048, 1920]
SIZES = [512, 512, 512, 384, 256, 128]

_nc_cache = None


def _build():
    nc = bacc.Bacc(None, target_bir_lowering=False)
    # tr: (A,B,A) k-stacking, rhs columns host-packed per piece so the
    # SPMD program uses identical packed offsets on both core-halves;
    # rows 48-127 repeat real data to pair with zero-padded weights
    tr_d = nc.declare_dram_parameter("tr", [128, PACK], BF16, isOutput=False)
    # tlb: (A,A,B) k-stacking, 6 x 128 piece rows (lhsT side)
    tlb_d = nc.declare_dram_parameter("tlb", [K3, 768], BF16, isOutput=False)
    out_d = nc.declare_dram_parameter("out", [K, 128, PACK], BF16,
                                      isOutput=True)

    with TileContext(nc) as tc:
        with (
            tc.tile_pool(name="vpool", bufs=1) as vpool,
            tc.tile_pool(name="stage", bufs=6) as stage,
            tc.tile_pool(name="psum", bufs=1, space=bass.MemorySpace.PSUM) as psum,
        ):
            tr = vpool.tile([128, PACK], BF16)
            tlb = vpool.tile([K3, 768], BF16)
            tr0 = vpool.tile([9, PACK], BF16)
            tlb0 = vpool.tile([9, 768], BF16)
            # early slices cover levels 0-2 (kk<=9); big loads follow.
            # tr is host-padded to 128 partitions (rows 48-127 repeat real
            # data) to pair with the zero-padded kk=128 weights below.
            nc.sync.dma_start(out=tlb0[:], in_=tlb_d[:9, :])
            nc.scalar.dma_start(out=tr0[:], in_=tr_d[:9, :])
            nc.sync.dma_start(out=tlb[:], in_=tlb_d[:])
            nc.scalar.dma_start(out=tr[:], in_=tr_d[:])

            # Zero-padded kk=128 weight tiles, one per level (l>=3): rows
            # 0..3(l+1)-1 are the real stacked operand, rows up to 127 are
            # zero, so a full-128 contraction gives the exact same result
            # while the PE array looks fully utilized (HAM warm-up => the
            # 2.4 GHz clock) and FWL kicks in (NumWeights==128). Built by
            # the otherwise-idle GpSimd from a zeroed template, staying
            # ahead of the PE's ~2 us/level pace.
            ztmpl = vpool.tile([128, 768], BF16)
            nc.gpsimd.memset(ztmpl[:], 0.0)
            pads = {}
            for l in range(3, K):
                pad = vpool.tile([128, 768], BF16, name=f"pad{l}")
                pads[l] = pad
                nc.gpsimd.tensor_copy(pad[:], ztmpl[:])
                nc.gpsimd.tensor_copy(pad[:3 * (l + 1), :], tlb[:3 * (l + 1), :])

            for l in range(K):
                kk = 128 if l >= 3 else 3 * (l + 1)
                lhs_t, rhs_t = (tlb0, tr0) if l <= 2 else (pads[l], tr)
                st = stage.tile([128, PACK], BF16, tag="st", name=f"st{l}")
                for s, (pair, off, w, nb) in enumerate(SEG):
                    ps = psum.tile([128, w], F32, tag=f"ps{s}",
                                   bufs=nb, name=f"ps{l}_{s}")
                    for p in pair:
                        o = POFFS[p] - off
                        nc.tensor.matmul(
                            ps[:, o:o + SIZES[p]],
                            lhsT=lhs_t[:kk, 128 * p:128 * (p + 1)],
                            rhs=rhs_t[:kk, POFFS[p]:POFFS[p] + SIZES[p]],
                            start=True,
                            stop=True,
                        )
                    # cast copy PSUM->SBUF, rotated across both engines
                    if (l + s) % 2 == 0:
                        nc.scalar.copy(st[:, off:off + w], ps[:])
                    else:
                        nc.vector.tensor_copy(st[:, off:off + w], ps[:])
                    if l == 0:  # ramp: store per segment
                        nc.sync.dma_start(
                            out=out_d[0, :, off:off + w],
                            in_=st[:, off:off + w],
                        )
                if l > 0:
                    nc.sync.dma_start(out=out_d[l], in_=st[:])

    nc.compile()
    return nc


def _get_nc():
    global _nc_cache
    if _nc_cache is None:
        _nc_cache = _build()
    return _nc_cache


def _prepare_in_maps(evecs: np.ndarray) -> list[dict]:
    poffs = POFFS
    in_maps = []
    for c in range(NCORES):
        b, h = divmod(c, 2)
        vt = np.ascontiguousarray(evecs[b, 0].T, dtype=np.float32)  # [K, N]
        a32 = vt.astype(BF16_NP).astype(np.float32)
        a = a32.astype(BF16_NP)                       # hi part
        bb = (vt - a32).astype(BF16_NP)               # lo part
        tr = np.empty((128, PACK), dtype=BF16_NP)
        tlb = np.empty((K3, 768), dtype=BF16_NP)
        for p, (i, c0, c1) in enumerate(PIECES[h]):
            cs = slice(poffs[p], poffs[p] + (c1 - c0))
            tr[0:K3:3, cs] = a[:, c0:c1]
            tr[1:K3:3, cs] = bb[:, c0:c1]
            tr[2:K3:3, cs] = a[:, c0:c1]
            rs = slice(128 * i, 128 * (i + 1))
            tlb[0::3, 128 * p:128 * (p + 1)] = a[:, rs]
            tlb[1::3, 128 * p:128 * (p + 1)] = a[:, rs]
            tlb[2::3, 128 * p:128 * (p + 1)] = bb[:, rs]
        # rows 48-127: repeat real data so the padded contraction's
        # streaming side keeps toggling (the matching weights are zero)
        tr[K3:2 * K3] = tr[:K3]
        tr[2 * K3:] = tr[:128 - 2 * K3]
        in_maps.append({"tr": tr, "tlb": tlb})
    return in_maps


def _assemble(results: list[dict]) -> np.ndarray:
    out = np.empty((B, K, N, N), dtype=np.float32)
    for b in range(B):
        for h in range(2):
            r = results[2 * b + h]["out"].astype(np.float32)  # [K,128,PACK]
            for p, (i, c0, c1) in enumerate(PIECES[h]):
                out[b, :, 128 * i:128 * (i + 1), c0:c1] = \
                    r[:, :, POFFS[p]:POFFS[p] + (c1 - c0)]
        # mirror the strictly-lower blocks from the computed upper ones
        for i in range(1, 8):
            out[b, :, 128 * i:128 * (i + 1), :128 * i] = np.swapaxes(
                out[b, :, :128 * i, 128 * i:128 * (i + 1)], -1, -2
            )
    return out.reshape(B, K * C, N, N)


def kernel(evecs) -> np.ndarray:
    evecs = np.asarray(evecs, dtype=np.float32)
    assert evecs.shape == (B, C, N, K), evecs.shape
    nc = _get_nc()
    in_maps = _prepare_in_maps(evecs)
    last_err = None
    for _attempt in range(3):
        try:
            r = run_bass_kernel_spmd(nc, in_maps, list(range(NCORES)))
            return _assemble(r.results)
        except Exception as e:  # transient NRT/device hiccups: retry
            last_err = e
    raise last_err


# revision 31
# speedup vs baseline: 1.7746x; 1.7746x over previous
"""Trainium2 Bass kernel for nn_ExpandEvecs.

Computes, for evecs [B=4, C=1, N=1024, K=16]:
    outers[b,k,i,j] = evecs[b,0,i,k] * evecs[b,0,j,k]
    cube = cumsum(outers, axis=k)  ->  [B, K, N, N]
i.e. cube[b,l] = V[:, :l+1] @ V[:, :l+1]^T  (Gram expansion per level).

Every level is SYMMETRIC, so the device only computes the upper
block-triangle (56% of the elements; diagonal 128-blocks in full) and
the host mirrors the strictly-lower blocks during unsharding. Output
is stored bf16 and upcast on the host (2.4e-3 max rel err vs the 2e-2
gate). Per-core HBM stores drop to 9 MiB (~26 us at the ~358 GB/s
HBM-per-core limit), PE columns and PSUM->SBUF copy work drop by the
same 2.3x vs the full-matrix version.

Sharding: 8 cores = 4 batches x 2 triangle-halves. The upper
block-triangle of each [1024,1024] level splits into six 128-row
pieces per core with IDENTICAL shapes on both cores (SPMD-safe):
sizes (512,512,512,384,256,128) columns. Piece p of a core is
(block-row i_p, cols c0_p:c1_p); the host knows the same table.

Per core, per level: 6 bf16 matmuls (one per piece, 2304 PE columns
total) using the A/B split trick (V = A + B with A = bf16(V),
B = bf16(V-A); lhsT/rhs partition-interleaved so AA^T+AB^T+BA^T comes
out of one matmul with contraction 3*(l+1); the dropped BB^T term is
~2^-18 relative). Pieces pack pairwise into three PSUM tiles
([128,1024], [128,896], [128,384]) so each level needs only three
PSUM->SBUF bf16 cast copies, alternating Vector/Scalar by level
parity. One contiguous 576 KiB store per level ([128, 2304] tile,
4.5 KiB runs per partition).
"""

import numpy as np
import ml_dtypes

import concourse.mybir as mybir
from concourse import bacc, bass
from concourse.tile import TileContext
from concourse.bass_utils import run_bass_kernel_spmd

B, C, N, K = 4, 1, 1024, 16
NCORES = 8
K3 = 3 * K             # stacked contraction partitions
PACK = 2304            # packed free dim per level (1024+896+384)

F32 = mybir.dt.float32
BF16 = mybir.dt.bfloat16
BF16_NP = ml_dtypes.bfloat16

# pieces per core-half: (block_row, col0, col1); identical shape lists
# (512,512,512,384,256,128) on both halves (SPMD-safe).
PIECES = [
    [(0, 0, 512), (0, 512, 1024), (4, 512, 1024),
     (1, 640, 1024), (2, 768, 1024), (3, 896, 1024)],
    [(1, 128, 640), (2, 256, 768), (3, 384, 896),
     (5, 640, 1024), (6, 768, 1024), (7, 896, 1024)],
]
# PSUM banks are 512 f32; tiles round up to whole banks and only 8
# exist. Five one-bank segments let the three 512-piece tags run with
# bufs=2 (6 banks) so no serial level-chain forms through them; the two
# small segments (384+128 packed, and 256) tolerate bufs=1 chains.
# SEG: (pieces, packed offset, width, bufs), in issue order (chained
# small segments first so their copies start earliest each level).
SEG = [
    ((3, 5), 1536, 512, 1),
    ((4,), 2048, 256, 1),
    ((0,), 0, 512, 2),
    ((1,), 512, 512, 2),
    ((2,), 1024, 512, 2),
]
# packed offset of each piece (piece index -> offset)
POFFS = [0, 512, 1024, 1536, 2048, 1920]
SIZES = [512, 512, 512, 384, 256, 128]

_nc_cache = None


def _build():
    nc = bacc.Bacc(None, target_bir_lowering=False)
    # tr: (A,B,A) k-stacking, rhs columns host-packed per piece so the
    # SPMD program uses identical packed offsets on both core-halves
    tr_d = nc.declare_dram_parameter("tr", [K3, PACK], BF16, isOutput=False)
    # tlb: (A,A,B) k-stacking, 6 x 128 piece rows (lhsT side)
    tlb_d = nc.declare_dram_parameter("tlb", [K3, 768], BF16, isOutput=False)
    out_d = nc.declare_dram_parameter("out", [K, 128, PACK], BF16,
                                      isOutput=True)

    with TileContext(nc) as tc:
        with (
            tc.tile_pool(name="vpool", bufs=1) as vpool,
            tc.tile_pool(name="stage", bufs=6) as stage,
            tc.tile_pool(name="psum", bufs=1, space=bass.MemorySpace.PSUM) as psum,
        ):
            tr = vpool.tile([K3, PACK], BF16)
            tlb = vpool.tile([K3, 768], BF16)
            tr0 = vpool.tile([9, PACK], BF16)
            tlb0 = vpool.tile([9, 768], BF16)
            # early slices cover levels 0-2 (kk<=9); big loads follow
            nc.sync.dma_start(out=tlb0[:], in_=tlb_d[:9, :])
            nc.scalar.dma_start(out=tr0[:], in_=tr_d[:9, :])
            nc.sync.dma_start(out=tlb[:], in_=tlb_d[:])
            nc.scalar.dma_start(out=tr[:], in_=tr_d[:])

            for l in range(K):
                kk = 3 * (l + 1)
                lhs_t, rhs_t = (tlb0, tr0) if l <= 2 else (tlb, tr)
                st = stage.tile([128, PACK], BF16, tag="st", name=f"st{l}")
                for s, (pair, off, w, nb) in enumerate(SEG):
                    ps = psum.tile([128, w], F32, tag=f"ps{s}",
                                   bufs=nb, name=f"ps{l}_{s}")
                    for p in pair:
                        o = POFFS[p] - off
                        nc.tensor.matmul(
                            ps[:, o:o + SIZES[p]],
                            lhsT=lhs_t[:kk, 128 * p:128 * (p + 1)],
                            rhs=rhs_t[:kk, POFFS[p]:POFFS[p] + SIZES[p]],
                            start=True,
                            stop=True,
                        )
                    # cast copy PSUM->SBUF, rotated across both engines
                    if (l + s) % 2 == 0:
                        nc.scalar.copy(st[:, off:off + w], ps[:])
                    else:
                        nc.vector.tensor_copy(st[:, off:off + w], ps[:])
                    if l == 0 or l == K - 1:
                        # ramp: store per segment to start the DMA stream
                        # early; last level likewise to shrink the tail
                        nc.sync.dma_start(
                            out=out_d[l, :, off:off + w],
                            in_=st[:, off:off + w],
                        )
                if 0 < l < K - 1:
                    nc.sync.dma_start(out=out_d[l], in_=st[:])

    nc.compile()
    return nc


def _get_nc():
    global _nc_cache
    if _nc_cache is None:
        _nc_cache = _build()
    return _nc_cache


def _prepare_in_maps(evecs: np.ndarray) -> list[dict]:
    poffs = POFFS
    in_maps = []
    for c in range(NCORES):
        b, h = divmod(c, 2)
        vt = np.ascontiguousarray(evecs[b, 0].T, dtype=np.float32)  # [K, N]
        a32 = vt.astype(BF16_NP).astype(np.float32)
        a = a32.astype(BF16_NP)                       # hi part
        bb = (vt - a32).astype(BF16_NP)               # lo part
        tr = np.empty((K3, PACK), dtype=BF16_NP)
        tlb = np.empty((K3, 768), dtype=BF16_NP)
        for p, (i, c0, c1) in enumerate(PIECES[h]):
            cs = slice(poffs[p], poffs[p] + (c1 - c0))
            tr[0::3, cs] = a[:, c0:c1]
            tr[1::3, cs] = bb[:, c0:c1]
            tr[2::3, cs] = a[:, c0:c1]
            rs = slice(128 * i, 128 * (i + 1))
            tlb[0::3, 128 * p:128 * (p + 1)] = a[:, rs]
            tlb[1::3, 128 * p:128 * (p + 1)] = a[:, rs]
            tlb[2::3, 128 * p:128 * (p + 1)] = bb[:, rs]
        in_maps.append({"tr": tr, "tlb": tlb})
    return in_maps


def _assemble(results: list[dict]) -> np.ndarray:
    out = np.empty((B, K, N, N), dtype=np.float32)
    for b in range(B):
        for h in range(2):
            r = results[2 * b + h]["out"].astype(np.float32)  # [K,128,PACK]
            for p, (i, c0, c1) in enumerate(PIECES[h]):
                out[b, :, 128 * i:128 * (i + 1), c0:c1] = \
                    r[:, :, POFFS[p]:POFFS[p] + (c1 - c0)]
        # mirror the strictly-lower blocks from the computed upper ones
        for i in range(1, 8):
            out[b, :, 128 * i:128 * (i + 1), :128 * i] = np.swapaxes(
                out[b, :, :128 * i, 128 * i:128 * (i + 1)], -1, -2
            )
    return out.reshape(B, K * C, N, N)


def kernel(evecs) -> np.ndarray:
    evecs = np.asarray(evecs, dtype=np.float32)
    assert evecs.shape == (B, C, N, K), evecs.shape
    nc = _get_nc()
    in_maps = _prepare_in_maps(evecs)
    last_err = None
    for _attempt in range(3):
        try:
            r = run_bass_kernel_spmd(nc, in_maps, list(range(NCORES)))
            return _assemble(r.results)
        except Exception as e:  # transient NRT/device hiccups: retry
            last_err = e
    raise last_err
